# revision 5
# baseline (speedup 1.0000x reference)
"""Trainium2 kernel for nn_CenterDisc (segment_reduce).

Computes: per-class (4 classes) mean of x rows (N=4096 rows of 64x512),
then mean pairwise Frobenius distance between the 4 class centers.

Strategy (data-parallel over N, 8 cores):
  - host: cast x to fp8e4m3 (final scalar tolerates it: rel err ~3e-4,
    gate is 2e-2), build one-hot(labels) per shard, shard rows 512/core
  - device: per-class partial sums via TensorE matmul in DoubleRow fp8
    mode (2 contraction rows per PE cell per cycle):
        sums[c, d] = sum_k onehot[k, c] * x[k, d]
    streaming 16 MB/core of fp8 x from HBM as 2 MB transfers split
    across both HWDGE rings (sync carries s-pair 0, scalar s-pair 1),
    fully prefetched; PSUM->SBUF copies on Vector; one-hot + output
    stores via GpSimd so the x stream is never blocked behind a copy.
  - host: add the 8 partial (4, 32768) bf16 sums, counts =
    bincount(labels), centers + pairwise norms (tiny) on host.
"""

import numpy as np
import ml_dtypes

import concourse.bass as bass
import concourse.tile as tile
from concourse import bacc, mybir
from concourse.bass import ts
from concourse.bass_utils import run_bass_kernel_spmd

# Problem shape (hardcoded per contract)
N, C, PDIM = 4096, 64, 512
D = C * PDIM           # 32768 features per row
NCLS = 4               # num classes
OHW = 16               # one-hot padded width (DoubleRow needs 16B k-step)
CORES = 8
R = N // CORES         # 512 rows per core
KP = 128               # rows per matmul subtile (partition dim)
S = R // KP            # 4 k-subtiles per core
SP = S // 2            # 2 DoubleRow pairs
SB = 8192              # feature columns per DMA stripe (2 MB per s-pair)
NS = D // SB           # 4 stripes
JB = 2048              # feature columns per compute block
NBLK = D // JB         # 16 compute blocks
MM = 512               # matmul moving free dim (PSUM bank = 512 fp32)
JS = JB // MM          # 4 matmul slices per block

_NC_CACHE = None


def _build_bass():
    nc = bacc.Bacc()
    f8 = mybir.dt.float8e4
    bf16 = mybir.dt.bfloat16
    x_in = nc.dram_tensor("x", [R, D], f8, kind="ExternalInput")
    oh_in = nc.dram_tensor("onehot", [R, OHW], f8, kind="ExternalInput")
    out = nc.dram_tensor("sums", [NCLS, D], bf16, kind="ExternalOutput")

    # tile[p, s, d] = x[s*128 + p, d]
    x_r = x_in[:, :].rearrange("(s p) d -> p s d", p=KP)     # (128, S, D)
    oh_r = oh_in[:, :].rearrange("(s p) c -> p s c", p=KP)   # (128, S, OHW)

    dr = mybir.MatmulPerfMode.DoubleRow

    with tile.TileContext(nc) as tc:
        with (
            tc.tile_pool(name="ohp", bufs=1) as ohp,
            tc.tile_pool(name="xp", bufs=4) as xp,
            tc.tile_pool(name="outp", bufs=4) as outp,
            tc.tile_pool(name="pp", bufs=2, space="PSUM") as pp,
        ):
            oht = ohp.tile([KP, S, OHW], f8, tag="oh")
            nc.gpsimd.dma_start(out=oht[:], in_=oh_r)

            xts = {}
            for sb in range(NS):
                # one 2 MB x load per stripe per DoubleRow s-pair:
                # sync ring carries s-pair 0, scalar ring s-pair 1.
                # Stripe 0's first compute block is loaded as its own
                # small piece so the first matmul starts early.
                for sp in range(SP):
                    xt = xp.tile([KP, 2, SB], f8, tag=f"x{sp}")
                    eng = nc.sync if sp == 0 else nc.scalar
                    src = x_r[:, 2 * sp:2 * sp + 2, sb * SB:(sb + 1) * SB]
                    if sb == 0:
                        eng.dma_start(out=xt[:, :, 0:JB], in_=src[:, :, 0:JB])
                        eng.dma_start(out=xt[:, :, JB:SB], in_=src[:, :, JB:SB])
                    else:
                        eng.dma_start(out=xt[:], in_=src)
                    xts[sp] = xt
                for cb in range(SB // JB):
                    blk = sb * (SB // JB) + cb
                    last = blk == NBLK - 1
                    pst = pp.tile([NCLS, JS, MM], mybir.dt.float32,
                                  tag="ps", name=f"ps{blk}")
                    for sp in range(SP):
                        for j in range(JS):
                            nc.tensor.matmul(
                                pst[:, j, :],
                                oht[:, 2 * sp:2 * sp + 2, 0:NCLS],
                                xts[sp][:, :, cb * JB + j * MM:
                                        cb * JB + (j + 1) * MM],
                                start=(sp == 0),
                                stop=(sp == SP - 1),
                                perf_mode=dr,
                            )
                    if not last:
                        ot = outp.tile([NCLS, JB], bf16, tag="ot")
                        nc.vector.tensor_copy(out=ot[:], in_=pst[:])
                        nc.gpsimd.dma_start(
                            out=out[:, blk * JB:(blk + 1) * JB], in_=ot[:])
                    else:
                        # split the final block's copy/store into halves on
                        # the (now idle) HWDGE rings to shorten the tail
                        for h, eng in ((0, nc.sync), (1, nc.scalar)):
                            oth = outp.tile([NCLS, JB // 2], bf16,
                                            tag=f"otl{h}")
                            nc.vector.tensor_copy(
                                out=oth[:], in_=pst[:, 2 * h:2 * h + 2, :])
                            eng.dma_start(
                                out=out[:, blk * JB + h * (JB // 2):
                                        blk * JB + (h + 1) * (JB // 2)],
                                in_=oth[:])
    nc.compile()
    return nc


def _get_nc():
    global _NC_CACHE
    if _NC_CACHE is None:
        _NC_CACHE = _build_bass()
    return _NC_CACHE


def _run(x, labels, trace=False, **spmd_kwargs):
    x = np.asarray(x, dtype=np.float32).reshape(N, D)
    x8 = x.astype(ml_dtypes.float8_e4m3)
    labels = np.asarray(labels).astype(np.int64)
    onehot = np.zeros((N, OHW), dtype=ml_dtypes.float8_e4m3)
    onehot[np.arange(N), labels] = 1.0

    in_maps = [
        {"x": x8[c * R:(c + 1) * R], "onehot": onehot[c * R:(c + 1) * R]}
        for c in range(CORES)
    ]
    nc = _get_nc()
    last_err = None
    for attempt in range(3):
        try:
            br = run_bass_kernel_spmd(nc, in_maps, core_ids=list(range(CORES)),
                                      trace=trace, **spmd_kwargs)
            break
        except Exception as e:  # transient device wedge (NRT_*) — retry
            last_err = e
            import time as _time
            _time.sleep(3.0)
    else:
        raise last_err

    sums = np.zeros((NCLS, D), dtype=np.float64)
    for r in br.results:
        sums += r["sums"].astype(np.float64)
    counts = np.bincount(labels, minlength=NCLS).astype(np.float64)
    safe = np.maximum(counts, 1.0)
    centers = sums / safe[:, None]                         # (NCLS, D)
    diffs = centers[:, None, :] - centers[None, :, :]      # (NCLS, NCLS, D)
    norms = np.sqrt(np.sum(diffs * diffs, axis=-1))        # (NCLS, NCLS)
    iu, ju = np.triu_indices(NCLS, k=1)
    distance = np.sum(norms[iu, ju]) / len(iu)
    return np.asarray(distance, dtype=np.float32), br


def kernel(x, labels):
    result, _ = _run(x, labels, trace=False)
    return result
